# revision 1
# baseline (speedup 1.0000x reference)
"""AFNO transformer block (LN -> AFNO2D -> +res -> LN -> MLP -> +res) on 8 trn2 cores.

Distribution: spatial (b,h)-rows sharded 8x for LN1/FFT-W and iFFT-W/LN2/MLP stages;
kw-spectral-columns sharded 8x for FFT-H/block-MM/iFFT-H middle stage; two
AllToAlls (bf16 spectra) swap between the shardings. FFTs are dense matmuls
against DFT matrices (only 46 of 91 W-modes survive; all H modes kept).
"""
import sys

sys.path.insert(0, "/opt/trn_rl_repo")
import numpy as np
from ml_dtypes import bfloat16

from concourse import bacc, tile, mybir
from concourse import bass_utils
from concourse.masks import make_identity

FP = mybir.dt.float32
BF = mybir.dt.bfloat16
AF = mybir.ActivationFunctionType
ALU = mybir.AluOpType

NCORES = 8
B, H, W, C = 2, 90, 180, 768
NB, BS = 8, 96
KW, KWP = 46, 48
ROWS = B * H            # 180
RPAD = 192              # padded rows
RPC = RPAD // NCORES    # 24 rows per core
KWL = KWP // NCORES     # 6 kw per core
HID = 4 * C             # 3072
LN_EPS = 1e-5
LAM = 0.01              # softshrink lambda
TOK = RPC * W           # 4320 tokens per core in stage B
WCH = [(0, 128), (128, 52)]   # w-axis partition chunks
CS = [(0, 384), (384, 384)]   # channel free-dim slices

_cache = {}


def _dft_consts():
    wi, ki, hi = np.arange(W), np.arange(KWP), np.arange(H)
    aw = 2 * np.pi * np.outer(wi, ki) / W
    FWr = np.cos(aw) / np.sqrt(W)
    FWi = -np.sin(aw) / np.sqrt(W)
    FWr[:, KW:] = 0.0
    FWi[:, KW:] = 0.0
    fw = np.concatenate([FWr, FWi], axis=1)          # [180, 96]
    ah = 2 * np.pi * np.outer(hi, hi) / H
    ehr = np.cos(ah) / np.sqrt(H)                    # symmetric
    ehi = -np.sin(ah) / np.sqrt(H)
    ck = np.where(ki == 0, 1.0, 2.0)
    CWr = (ck[:, None] * np.cos(aw.T)) / np.sqrt(W)  # [48, 180]
    CWi = (-ck[:, None] * np.sin(aw.T)) / np.sqrt(W)
    CWi[0, :] = 0.0
    CWr[KW:, :] = 0.0
    CWi[KW:, :] = 0.0
    cw = np.concatenate([CWr, CWi], axis=0)          # [96, 180]
    b16 = lambda a: np.ascontiguousarray(a).astype(bfloat16)
    return dict(fw=b16(fw), ehr=b16(ehr), ehi=b16(ehi), ehin=b16(-ehi),
                eir=b16(ehr), eii=b16(-ehi), eiin=b16(ehi), cw=b16(cw))


def _build():
    nc = bacc.Bacc("TRN2", target_bir_lowering=False, debug=False,
                   num_devices=NCORES)

    def din(name, shape, dt=FP):
        return nc.dram_tensor(name, list(shape), dt, kind="ExternalInput").ap()

    x_sh = din("x_sh", [RPC, W, C])
    fw_d = din("fw_d", [W, 2 * KWP], BF)
    ehr_d = din("ehr_d", [H, H], BF)
    ehi_d = din("ehi_d", [H, H], BF)
    ehin_d = din("ehin_d", [H, H], BF)
    eir_d = din("eir_d", [H, H], BF)
    eii_d = din("eii_d", [H, H], BF)
    eiin_d = din("eiin_d", [H, H], BF)
    cw_d = din("cw_d", [2 * KWP, W], BF)
    blk1_d = din("blk1_d", [NB, 3, BS, BS], BF)   # [w1r, w1i, -w1i] per block
    blk2_d = din("blk2_d", [NB, 3, BS, BS], BF)
    b1_d = din("b1_d", [2, NB, BS])
    b2_d = din("b2_d", [2, NB, BS])
    fc1w_d = din("fc1w_d", [C, HID], BF)
    fc2w_d = din("fc2w_d", [HID, C], BF)
    fc1b_d = din("fc1b_d", [1, HID])
    fc2b_d = din("fc2b_d", [1, C])
    n1w_d = din("n1w_d", [1, C])
    n1b_d = din("n1b_d", [1, C])
    n2w_d = din("n2w_d", [1, C])
    n2b_d = din("n2b_d", [1, C])

    out_sh = nc.dram_tensor("out_sh", [RPC, W, C], FP, kind="ExternalOutput").ap()

    s_dram = nc.dram_tensor("s_dram", [RPC, W, C], FP).ap()
    h_dram = nc.dram_tensor("h_dram", [RPC, W, C], FP).ap()
    a2a1_in = nc.dram_tensor("a2a1_in", [NCORES, RPC, KWL, 2, C], BF).ap()
    a2a1_out = nc.dram_tensor("a2a1_out", [NCORES, RPC, KWL, 2, C], BF).ap()
    a2a2_in = nc.dram_tensor("a2a2_in", [NCORES, RPC, KWL, 2, C], BF).ap()
    a2a2_out = nc.dram_tensor("a2a2_out", [NCORES, RPC, KWL, 2, C], BF).ap()

    rg = [list(range(NCORES))]

    with tile.TileContext(nc) as tc:
        with tc.tile_pool(name="cpool", bufs=1) as cp:
            # ---- small constants resident whole kernel ----
            ident = cp.tile([128, 128], BF)
            make_identity(nc, ident[:])
            fw_a = cp.tile([128, 2 * KWP], BF)
            fw_b = cp.tile([52, 2 * KWP], BF)
            nc.sync.dma_start(out=fw_a[:], in_=fw_d[0:128, :])
            nc.sync.dma_start(out=fw_b[:], in_=fw_d[128:180, :])
            ehr = cp.tile([H, H], BF); nc.sync.dma_start(out=ehr[:], in_=ehr_d[:])
            ehi = cp.tile([H, H], BF); nc.sync.dma_start(out=ehi[:], in_=ehi_d[:])
            ehin = cp.tile([H, H], BF); nc.sync.dma_start(out=ehin[:], in_=ehin_d[:])
            eir = cp.tile([H, H], BF); nc.sync.dma_start(out=eir[:], in_=eir_d[:])
            eii = cp.tile([H, H], BF); nc.sync.dma_start(out=eii[:], in_=eii_d[:])
            eiin = cp.tile([H, H], BF); nc.sync.dma_start(out=eiin[:], in_=eiin_d[:])
            cw = cp.tile([2 * KWP, W], BF); nc.sync.dma_start(out=cw[:], in_=cw_d[:])
            blk1 = cp.tile([BS, NB, 3, BS], BF)
            blk2 = cp.tile([BS, NB, 3, BS], BF)
            nc.sync.dma_start(out=blk1[:], in_=blk1_d.rearrange("k v i o -> i k v o"))
            nc.sync.dma_start(out=blk2[:], in_=blk2_d.rearrange("k v i o -> i k v o"))
            b1c = cp.tile([BS, 2, NB], FP)
            b2c = cp.tile([BS, 2, NB], FP)
            nc.sync.dma_start(out=b1c[:], in_=b1_d.rearrange("v k o -> o v k"))
            nc.sync.dma_start(out=b2c[:], in_=b2_d.rearrange("v k o -> o v k"))
            fc1b = cp.tile([128, 24], FP)
            fc2b = cp.tile([128, 6], FP)
            nc.sync.dma_start(out=fc1b[:], in_=fc1b_d.rearrange("x (m p) -> p (x m)", p=128))
            nc.sync.dma_start(out=fc2b[:], in_=fc2b_d.rearrange("x (m p) -> p (x m)", p=128))

            # ================= STAGE A: LN1 + FFT-W per row =================
            with (
                tc.tile_pool(name="sa", bufs=3) as sa,
                tc.tile_pool(name="na", bufs=1) as na,
                tc.tile_pool(name="saps", bufs=2, space="PSUM") as saps,
            ):
                n1w_r = na.tile([128, C], FP)
                n1b_r = na.tile([128, C], FP)
                nc.sync.dma_start(out=n1w_r[:], in_=n1w_d[:].partition_broadcast(128))
                nc.sync.dma_start(out=n1b_r[:], in_=n1b_d[:].partition_broadcast(128))
                for r in range(RPC):
                    h1bf = []
                    for ci, (w0, wn) in enumerate(WCH):
                        xa = sa.tile([128, C], FP, tag=f"xa{ci}", name=f"xa_{r}_{ci}")
                        nc.sync.dma_start(out=xa[:wn], in_=x_sh[r, w0:w0 + wn, :])
                        st = sa.tile([128, 2, 6], FP, tag=f"st{ci}", name=f"st_{r}_{ci}")
                        nc.vector.bn_stats(st[:wn, 0, :], xa[:wn, 0:384])
                        nc.vector.bn_stats(st[:wn, 1, :], xa[:wn, 384:768])
                        ag = sa.tile([128, 2], FP, tag=f"ag{ci}", name=f"ag_{r}_{ci}")
                        nc.vector.bn_aggr(ag[:wn], st[:wn])
                        ve = sa.tile([128, 1], FP, tag=f"ve{ci}", name=f"ve_{r}_{ci}")
                        nc.vector.tensor_scalar_add(ve[:wn], ag[:wn, 1:2], LN_EPS)
                        sd = sa.tile([128, 1], FP, tag=f"sd{ci}", name=f"sd_{r}_{ci}")
                        nc.scalar.activation(sd[:wn], ve[:wn], AF.Sqrt)
                        rs = sa.tile([128, 1], FP, tag=f"rs{ci}", name=f"rs_{r}_{ci}")
                        nc.vector.reciprocal(rs[:wn], sd[:wn])
                        t_ = sa.tile([128, C], FP, tag=f"t{ci}", name=f"t_{r}_{ci}")
                        nc.vector.tensor_scalar(t_[:wn], xa[:wn], ag[:wn, 0:1], rs[:wn],
                                                ALU.subtract, ALU.mult)
                        m1 = sa.tile([128, C], FP, tag=f"m1{ci}", name=f"m1_{r}_{ci}")
                        nc.vector.tensor_mul(m1[:wn], t_[:wn], n1w_r[:wn])
                        xb = sa.tile([128, C], FP, tag=f"xb{ci}", name=f"xb_{r}_{ci}")
                        nc.vector.tensor_add(xb[:wn], xa[:wn], n1b_r[:wn])
                        sv = sa.tile([128, C], FP, tag=f"sv{ci}", name=f"sv_{r}_{ci}")
                        nc.vector.tensor_add(sv[:wn], m1[:wn], xb[:wn])
                        nc.sync.dma_start(out=s_dram[r, w0:w0 + wn, :], in_=sv[:wn])
                        hb = sa.tile([128, C], BF, tag=f"hb{ci}", name=f"hb_{r}_{ci}")
                        nc.vector.tensor_add(hb[:wn], m1[:wn], n1b_r[:wn])
                        h1bf.append(hb)
                    for si, (c0, cn) in enumerate(CS):
                        psy = saps.tile([2 * KWP, 384], FP, tag="psy", name=f"psy_{r}_{si}")
                        nc.tensor.matmul(psy[:], fw_a[:], h1bf[0][:, c0:c0 + cn],
                                         start=True, stop=False)
                        nc.tensor.matmul(psy[:], fw_b[:], h1bf[1][0:52, c0:c0 + cn],
                                         start=False, stop=True)
                        yb = sa.tile([2 * KWP, 384], BF, tag="yb", name=f"yb_{r}_{si}")
                        nc.scalar.copy(yb[:], psy[:])
                        for comp in range(2):
                            dst = a2a1_in[:, r, :, comp, c0:c0 + cn]
                            nc.sync.dma_start(out=dst,
                                              in_=yb[comp * KWP:(comp + 1) * KWP, :])

            nc.gpsimd.collective_compute(
                "AllToAll", ALU.bypass, replica_groups=rg,
                ins=[a2a1_in[:]], outs=[a2a1_out[:]])

            # ================= MIDDLE: FFT-H, block MM, iFFT-H =================
            v1 = a2a1_out.rearrange("s r kwl comp c -> (s r) kwl comp c")
            v2in = a2a2_in.rearrange("s r kwl comp c -> (s r) kwl comp c")
            NTOK = B * KWL * H  # 1080 spectral tokens per core
            with tc.tile_pool(name="mz", bufs=1) as mz:
                zsb = mz.tile([BS, NB, 2, NTOK], BF)   # [96, k, comp, (b kwl kh)]
                osb = mz.tile([H, B, KWL, 2, NB, BS], BF)
                with (
                    tc.tile_pool(name="m1", bufs=2) as m1p,
                    tc.tile_pool(name="m1ps", bufs=2, space="PSUM") as m1ps,
                ):
                    for b in range(B):
                        ybt = m1p.tile([H, KWL, 2, C], BF, tag="ybt", name=f"ybt_{b}")
                        nc.sync.dma_start(out=ybt[:], in_=v1[b * H:(b + 1) * H])
                        for kwl in range(KWL):
                            for si, (c0, cn) in enumerate(CS):
                                pr = m1ps.tile([H, 384], FP, tag="pr", name=f"pr_{b}_{kwl}_{si}")
                                pi = m1ps.tile([H, 384], FP, tag="pi", name=f"pi_{b}_{kwl}_{si}")
                                rr = ybt[:, kwl, 0, c0:c0 + cn]
                                ri = ybt[:, kwl, 1, c0:c0 + cn]
                                nc.tensor.matmul(pr[:], ehr[:], rr, start=True, stop=False)
                                nc.tensor.matmul(pr[:], ehin[:], ri, start=False, stop=True)
                                nc.tensor.matmul(pi[:], ehi[:], rr, start=True, stop=False)
                                nc.tensor.matmul(pi[:], ehr[:], ri, start=False, stop=True)
                                tsb = m1p.tile([H, 2, 384], BF, tag="tsb",
                                               name=f"tsb_{b}_{kwl}_{si}")
                                nc.scalar.copy(tsb[:, 0], pr[:])
                                nc.scalar.copy(tsb[:, 1], pi[:])
                                for comp in range(2):
                                    for cb in range(4):
                                        k = si * 4 + cb
                                        pz = m1ps.tile([BS, H], BF, tag="pz",
                                                       name=f"pz_{b}_{kwl}_{si}_{comp}_{cb}")
                                        nc.tensor.transpose(
                                            pz[:], tsb[:, comp, cb * BS:(cb + 1) * BS],
                                            ident[0:H, 0:H])
                                        tk0 = (b * KWL + kwl) * H
                                        nc.vector.tensor_copy(
                                            zsb[:, k, comp, tk0:tk0 + H], pz[:])
                with (
                    tc.tile_pool(name="m2", bufs=2) as m2p,
                    tc.tile_pool(name="m2ps", bufs=2, space="PSUM") as m2ps,
                ):
                    BCH = [(0, 512), (512, 512), (1024, NTOK - 1024)]
                    for k in range(NB):
                        for t0, tn in BCH:
                            p1r = m2ps.tile([BS, 512], FP, tag="p1r", name=f"p1r_{k}_{t0}")
                            p1i = m2ps.tile([BS, 512], FP, tag="p1i", name=f"p1i_{k}_{t0}")
                            zr = zsb[:, k, 0, t0:t0 + tn]
                            zi = zsb[:, k, 1, t0:t0 + tn]
                            nc.tensor.matmul(p1r[:, :tn], blk1[:, k, 0], zr, start=True, stop=False)
                            nc.tensor.matmul(p1r[:, :tn], blk1[:, k, 2], zi, start=False, stop=True)
                            nc.tensor.matmul(p1i[:, :tn], blk1[:, k, 1], zr, start=True, stop=False)
                            nc.tensor.matmul(p1i[:, :tn], blk1[:, k, 0], zi, start=False, stop=True)
                            o1r = m2p.tile([BS, 512], BF, tag="o1r", name=f"o1r_{k}_{t0}")
                            o1i = m2p.tile([BS, 512], BF, tag="o1i", name=f"o1i_{k}_{t0}")
                            nc.scalar.activation(o1r[:, :tn], p1r[:, :tn], AF.Relu,
                                                 bias=b1c[:, 0, k:k + 1])
                            nc.scalar.activation(o1i[:, :tn], p1i[:, :tn], AF.Relu,
                                                 bias=b1c[:, 1, k:k + 1])
                            p2r = m2ps.tile([BS, 512], FP, tag="p2r", name=f"p2r_{k}_{t0}")
                            p2i = m2ps.tile([BS, 512], FP, tag="p2i", name=f"p2i_{k}_{t0}")
                            nc.tensor.matmul(p2r[:, :tn], blk2[:, k, 0], o1r[:, :tn], start=True, stop=False)
                            nc.tensor.matmul(p2r[:, :tn], blk2[:, k, 2], o1i[:, :tn], start=False, stop=True)
                            nc.tensor.matmul(p2i[:, :tn], blk2[:, k, 1], o1r[:, :tn], start=True, stop=False)
                            nc.tensor.matmul(p2i[:, :tn], blk2[:, k, 0], o1i[:, :tn], start=False, stop=True)
                            for comp, ps2 in ((0, p2r), (1, p2i)):
                                tb = m2p.tile([BS, 512], FP, tag=f"tb{comp}",
                                              name=f"tb_{k}_{t0}_{comp}")
                                nc.vector.tensor_scalar_add(tb[:, :tn], ps2[:, :tn],
                                                            b2c[:, comp, k:k + 1])
                                cl = m2p.tile([BS, 512], FP, tag=f"cl{comp}",
                                              name=f"cl_{k}_{t0}_{comp}")
                                nc.vector.tensor_scalar(cl[:, :tn], tb[:, :tn], -LAM, LAM,
                                                        ALU.max, ALU.min)
                                # softshrink written back in place of zsb
                                nc.vector.tensor_sub(zsb[:, k, comp, t0:t0 + tn],
                                                     tb[:, :tn], cl[:, :tn])
                with (
                    tc.tile_pool(name="m3", bufs=2) as m3p,
                    tc.tile_pool(name="m3ps", bufs=2, space="PSUM") as m3ps,
                ):
                    # transpose back to [kh, c] then iFFT-H, then bounce out
                    for b in range(B):
                        for kwl in range(KWL):
                            tk0 = (b * KWL + kwl) * H
                            for k in range(NB):
                                for comp in range(2):
                                    po = m3ps.tile([H, BS], BF, tag="po",
                                                   name=f"po_{b}_{kwl}_{k}_{comp}")
                                    nc.tensor.transpose(
                                        po[:], zsb[:, k, comp, tk0:tk0 + H],
                                        ident[0:BS, 0:BS])
                                    nc.vector.tensor_copy(osb[:, b, kwl, comp, k, :], po[:])
                            for si, (c0, cn) in enumerate(CS):
                                ks = si * 4
                                orr = osb[:, b, kwl, 0, ks:ks + 4, :]
                                ori = osb[:, b, kwl, 1, ks:ks + 4, :]
                                pur = m3ps.tile([H, 384], FP, tag="pur",
                                                name=f"pur_{b}_{kwl}_{si}")
                                pui = m3ps.tile([H, 384], FP, tag="pui",
                                                name=f"pui_{b}_{kwl}_{si}")
                                nc.tensor.matmul(pur[:], eir[:], orr, start=True, stop=False)
                                nc.tensor.matmul(pur[:], eiin[:], ori, start=False, stop=True)
                                nc.tensor.matmul(pui[:], eii[:], orr, start=True, stop=False)
                                nc.tensor.matmul(pui[:], eir[:], ori, start=False, stop=True)
                                ub = m3p.tile([H, 2, 384], BF, tag="ub",
                                              name=f"ub_{b}_{kwl}_{si}")
                                nc.scalar.copy(ub[:, 0], pur[:])
                                nc.scalar.copy(ub[:, 1], pui[:])
                                nc.sync.dma_start(
                                    out=v2in[b * H:(b + 1) * H, kwl, 0, c0:c0 + cn],
                                    in_=ub[:, 0])
                                nc.sync.dma_start(
                                    out=v2in[b * H:(b + 1) * H, kwl, 1, c0:c0 + cn],
                                    in_=ub[:, 1])

            nc.gpsimd.collective_compute(
                "AllToAll", ALU.bypass, replica_groups=rg,
                ins=[a2a2_in[:]], outs=[a2a2_out[:]])

            # ================= STAGE B: iFFT-W + LN2 + MLP =================
            with tc.tile_pool(name="sbB", bufs=1) as sbB:
                mlp_in = sbB.tile([128, 6, TOK], BF)
                with (
                    tc.tile_pool(name="b1p", bufs=3) as b1p,
                    tc.tile_pool(name="nb", bufs=1) as nb_,
                    tc.tile_pool(name="b1ps", bufs=2, space="PSUM") as b1ps,
                ):
                    n2w_r = nb_.tile([128, C], FP)
                    n2b_r = nb_.tile([128, C], FP)
                    nc.sync.dma_start(out=n2w_r[:], in_=n2w_d[:].partition_broadcast(128))
                    nc.sync.dma_start(out=n2b_r[:], in_=n2b_d[:].partition_broadcast(128))
                    for r in range(RPC):
                        usb = b1p.tile([2 * KWP, C], BF, tag="usb", name=f"usb_{r}")
                        for comp in range(2):
                            nc.sync.dma_start(
                                out=usb[comp * KWP:(comp + 1) * KWP, :],
                                in_=a2a2_out[:, r, :, comp, :])
                        for ci, (w0, wn) in enumerate(WCH):
                            ht = b1p.tile([128, C], FP, tag=f"ht{ci}", name=f"ht_{r}_{ci}")
                            stile = b1p.tile([128, C], FP, tag=f"stl{ci}", name=f"stl_{r}_{ci}")
                            nc.sync.dma_start(out=stile[:wn], in_=s_dram[r, w0:w0 + wn, :])
                            for si, (c0, cn) in enumerate(CS):
                                py = b1ps.tile([128, 384], FP, tag="py",
                                               name=f"py_{r}_{ci}_{si}")
                                nc.tensor.matmul(py[:wn], cw[:, w0:w0 + wn],
                                                 usb[:, c0:c0 + cn], start=True, stop=True)
                                nc.vector.tensor_add(ht[:wn, c0:c0 + cn], py[:wn],
                                                     stile[:wn, c0:c0 + cn])
                            nc.sync.dma_start(out=h_dram[r, w0:w0 + wn, :], in_=ht[:wn])
                            # LN2
                            st = b1p.tile([128, 2, 6], FP, tag=f"st{ci}", name=f"bst_{r}_{ci}")
                            nc.vector.bn_stats(st[:wn, 0, :], ht[:wn, 0:384])
                            nc.vector.bn_stats(st[:wn, 1, :], ht[:wn, 384:768])
                            ag = b1p.tile([128, 2], FP, tag=f"ag{ci}", name=f"bag_{r}_{ci}")
                            nc.vector.bn_aggr(ag[:wn], st[:wn])
                            ve = b1p.tile([128, 1], FP, tag=f"ve{ci}", name=f"bve_{r}_{ci}")
                            nc.vector.tensor_scalar_add(ve[:wn], ag[:wn, 1:2], LN_EPS)
                            sd = b1p.tile([128, 1], FP, tag=f"sd{ci}", name=f"bsd_{r}_{ci}")
                            nc.scalar.activation(sd[:wn], ve[:wn], AF.Sqrt)
                            rs = b1p.tile([128, 1], FP, tag=f"rs{ci}", name=f"brs_{r}_{ci}")
                            nc.vector.reciprocal(rs[:wn], sd[:wn])
                            t2 = b1p.tile([128, C], FP, tag=f"t2{ci}", name=f"bt2_{r}_{ci}")
                            nc.vector.tensor_scalar(t2[:wn], ht[:wn], ag[:wn, 0:1], rs[:wn],
                                                    ALU.subtract, ALU.mult)
                            m2_ = b1p.tile([128, C], FP, tag=f"m2{ci}", name=f"bm2_{r}_{ci}")
                            nc.vector.tensor_mul(m2_[:wn], t2[:wn], n2w_r[:wn])
                            h2 = b1p.tile([128, C], BF, tag=f"h2{ci}", name=f"bh2_{r}_{ci}")
                            nc.vector.tensor_add(h2[:wn], m2_[:wn], n2b_r[:wn])
                            # transpose h2 [wn, 768] -> mlp_in[:, kc, r*W + w0 : +wn]
                            tok0 = r * W + w0
                            for kc in range(6):
                                pt = b1ps.tile([128, 128], BF, tag="pt",
                                               name=f"pt_{r}_{ci}_{kc}")
                                nc.tensor.transpose(pt[:, :wn],
                                                    h2[:wn, kc * 128:(kc + 1) * 128],
                                                    ident[:wn, :wn])
                                nc.vector.tensor_copy(mlp_in[:, kc, tok0:tok0 + wn],
                                                      pt[:, :wn])
                # ---- MLP over token chunks ----
                hv = h_dram.rearrange("r w c -> (r w) c")
                ov = out_sh.rearrange("r w c -> (r w) c")
                with (
                    tc.tile_pool(name="wpB", bufs=1) as wpB,
                    tc.tile_pool(name="b2p", bufs=2) as b2p,
                    tc.tile_pool(name="gp", bufs=1) as gp,
                    tc.tile_pool(name="b2ps", bufs=2, space="PSUM") as b2ps,
                    tc.tile_pool(name="b2psg", bufs=3, space="PSUM") as b2psg,
                ):
                    fc1w = wpB.tile([128, 6, 24, 128], BF)
                    fc2w = wpB.tile([128, 24, 6, 128], BF)
                    nc.sync.dma_start(out=fc1w[:], in_=fc1w_d.rearrange(
                        "(kc p) (m n) -> p kc m n", p=128, n=128))
                    nc.sync.dma_start(out=fc2w[:], in_=fc2w_d.rearrange(
                        "(kc p) (m n) -> p kc m n", p=128, n=128))
                    for t0 in range(0, TOK, 512):
                        tn = min(512, TOK - t0)
                        gsb = gp.tile([128, 24, 512], BF, tag="gsb", name=f"gsb_{t0}")
                        for m in range(24):
                            pg = b2psg.tile([128, 512], FP, tag="pg", name=f"pg_{t0}_{m}")
                            for kc in range(6):
                                nc.tensor.matmul(pg[:, :tn], fc1w[:, kc, m],
                                                 mlp_in[:, kc, t0:t0 + tn],
                                                 start=(kc == 0), stop=(kc == 5))
                            nc.scalar.activation(gsb[:, m, :tn], pg[:, :tn], AF.Gelu,
                                                 bias=fc1b[:, m:m + 1])
                        fsb = []
                        for mo in range(6):
                            po = b2ps.tile([128, 512], FP, tag="pofc2",
                                           name=f"po_{t0}_{mo}")
                            for kc in range(24):
                                nc.tensor.matmul(po[:, :tn], fc2w[:, kc, mo],
                                                 gsb[:, kc, :tn],
                                                 start=(kc == 0), stop=(kc == 23))
                            fo = b2p.tile([128, 512], BF, tag=f"fo{mo}",
                                          name=f"fo_{t0}_{mo}")
                            nc.vector.tensor_scalar_add(fo[:, :tn], po[:, :tn],
                                                        fc2b[:, mo:mo + 1])
                            fsb.append(fo)
                        for ts0 in range(0, tn, 128):
                            tsn = min(128, tn - ts0)
                            hht = b2p.tile([128, C], FP, tag="hht", name=f"hht_{t0}_{ts0}")
                            nc.sync.dma_start(out=hht[:tsn],
                                              in_=hv[t0 + ts0:t0 + ts0 + tsn, :])
                            outt = b2p.tile([128, C], FP, tag="outt", name=f"outt_{t0}_{ts0}")
                            for mo in range(6):
                                ptt = b2ps.tile([128, 128], BF, tag="ptt",
                                                name=f"ptt_{t0}_{ts0}_{mo}")
                                nc.tensor.transpose(ptt[:tsn, :],
                                                    fsb[mo][:, ts0:ts0 + tsn],
                                                    ident[:, :])
                                nc.vector.tensor_add(outt[:tsn, mo * 128:(mo + 1) * 128],
                                                     ptt[:tsn, :],
                                                     hht[:tsn, mo * 128:(mo + 1) * 128])
                            nc.sync.dma_start(out=ov[t0 + ts0:t0 + ts0 + tsn, :],
                                              in_=outt[:tsn])
    nc.compile()
    return nc


def _prep_inputs(inputs):
    consts = _dft_consts()
    x = np.asarray(inputs["x"], np.float32)
    xp = np.zeros((RPAD, W, C), np.float32)
    xp[:ROWS] = x.reshape(ROWS, W, C)
    w1 = np.asarray(inputs["w1"], np.float32)
    w2 = np.asarray(inputs["w2"], np.float32)
    blk1 = np.stack([w1[0], w1[1], -w1[1]], axis=1).astype(bfloat16)  # [8,3,96,96]
    blk2 = np.stack([w2[0], w2[1], -w2[1]], axis=1).astype(bfloat16)
    f32 = lambda k: np.ascontiguousarray(np.asarray(inputs[k], np.float32))
    common = dict(
        fw_d=consts["fw"], ehr_d=consts["ehr"], ehi_d=consts["ehi"],
        ehin_d=consts["ehin"], eir_d=consts["eir"], eii_d=consts["eii"],
        eiin_d=consts["eiin"], cw_d=consts["cw"],
        blk1_d=blk1, blk2_d=blk2,
        b1_d=f32("b1"), b2_d=f32("b2"),
        fc1w_d=f32("fc1_w").astype(bfloat16), fc2w_d=f32("fc2_w").astype(bfloat16),
        fc1b_d=f32("fc1_b").reshape(1, HID), fc2b_d=f32("fc2_b").reshape(1, C),
        n1w_d=f32("norm1_w").reshape(1, C), n1b_d=f32("norm1_b").reshape(1, C),
        n2w_d=f32("norm2_w").reshape(1, C), n2b_d=f32("norm2_b").reshape(1, C),
    )
    in_maps = []
    for q in range(NCORES):
        m = dict(common)
        m["x_sh"] = np.ascontiguousarray(xp[q * RPC:(q + 1) * RPC])
        in_maps.append(m)
    return in_maps


last_exec_time_ns = None


def kernel(**inputs):
    global last_exec_time_ns
    bass_utils.upload_artifacts = lambda tmpdir: ""  # avoid bucket upload hang under trace
    if "nc" not in _cache:
        _cache["nc"] = _build()
    nc = _cache["nc"]
    in_maps = _prep_inputs(inputs)
    res = bass_utils.run_bass_kernel_spmd(
        nc, in_maps, core_ids=list(range(NCORES)),
        trace=bool(int(__import__("os").environ.get("KERNEL_TRACE", "0"))))
    last_exec_time_ns = res.exec_time_ns
    out = np.concatenate([res.results[q]["out_sh"] for q in range(NCORES)], axis=0)
    return np.ascontiguousarray(out[:ROWS].reshape(B, H, W, C))



# revision 5
# speedup vs baseline: 1.1320x; 1.1320x over previous
"""AFNO transformer block (LN -> AFNO2D -> +res -> LN -> MLP -> +res) on 8 trn2 cores.

Distribution: spatial (b,h)-rows sharded 8x for LN1/FFT-W and iFFT-W/LN2/MLP stages;
kw-spectral-columns sharded 8x for FFT-H/block-MM/iFFT-H middle stage; two
AllToAlls (bf16 spectra) swap between the shardings. FFTs are dense matmuls
against DFT matrices (only 46 of 91 W-modes survive; all H modes kept).

v2: LN chains fused into scalar_tensor_tensor pairs (bf16), residual adds on
GpSimd, FFT-H and block-MM-2 run data-stationary so spectra land in the layout
the next stage needs (no PE transposes in the middle), fc2 runs data-stationary
(no output transposes), LN2 affine folded into fc1 weights host-side, residual
streams in bf16.
"""
import sys

sys.path.insert(0, "/opt/trn_rl_repo")
import numpy as np
from ml_dtypes import bfloat16

from concourse import bacc, tile, mybir
from concourse import bass_utils
from concourse.masks import make_identity

FP = mybir.dt.float32
BF = mybir.dt.bfloat16
AF = mybir.ActivationFunctionType
ALU = mybir.AluOpType

NCORES = 8
B, H, W, C = 2, 90, 180, 768
NB, BS = 8, 96
KW, KWP = 46, 48
ROWS = B * H            # 180
RPAD = 192              # padded rows
RPC = RPAD // NCORES    # 24 rows per core
KWL = KWP // NCORES     # 6 kw per core
HID = 4 * C             # 3072
LN_EPS = 1e-5
LAM = 0.01              # softshrink lambda
TOK = RPC * W           # 4320 tokens per core in stage B
NTOK = B * KWL * H      # 1080 spectral tokens per core in the middle
WCH = [(0, 128), (128, 52)]   # w-axis partition chunks
CS = [(0, 384), (384, 384)]   # channel free-dim slices
BCH = [(0, 512), (512, 512), (1024, NTOK - 1024)]

_cache = {}


def _dft_consts():
    wi, ki, hi = np.arange(W), np.arange(KWP), np.arange(H)
    aw = 2 * np.pi * np.outer(wi, ki) / W
    FWr = np.cos(aw) / np.sqrt(W)
    FWi = -np.sin(aw) / np.sqrt(W)
    FWr[:, KW:] = 0.0
    FWi[:, KW:] = 0.0
    fw = np.concatenate([FWr, FWi], axis=1)          # [180, 96]
    ah = 2 * np.pi * np.outer(hi, hi) / H
    ehr = np.cos(ah) / np.sqrt(H)                    # symmetric
    ehi = -np.sin(ah) / np.sqrt(H)
    fht = np.concatenate([ehr, ehi], axis=1)         # [90, 180]
    fht2 = np.concatenate([-ehi, ehr], axis=1)       # [90, 180]
    ck = np.where(ki == 0, 1.0, 2.0)
    CWr = (ck[:, None] * np.cos(aw.T)) / np.sqrt(W)  # [48, 180]
    CWi = (-ck[:, None] * np.sin(aw.T)) / np.sqrt(W)
    CWi[0, :] = 0.0
    CWr[KW:, :] = 0.0
    CWi[KW:, :] = 0.0
    cw = np.concatenate([CWr, CWi], axis=0)          # [96, 180]
    b16 = lambda a: np.ascontiguousarray(a).astype(bfloat16)
    return dict(fw=b16(fw), fht=b16(fht), fht2=b16(fht2), cw=b16(cw))


def _build():
    nc = bacc.Bacc("TRN2", target_bir_lowering=False, debug=False,
                   num_devices=NCORES)

    def din(name, shape, dt=FP):
        return nc.dram_tensor(name, list(shape), dt, kind="ExternalInput").ap()

    x_sh = din("x_sh", [RPC, W, C])
    fw_d = din("fw_d", [W, 2 * KWP], BF)
    fht_d = din("fht_d", [H, 2 * H], BF)
    fht2_d = din("fht2_d", [H, 2 * H], BF)
    cw_d = din("cw_d", [2 * KWP, W], BF)
    blk1_d = din("blk1_d", [NB, 3, BS, BS], BF)   # [w1r, w1i, -w1i] per block
    w2p_d = din("w2p_d", [BS + 1, NB, 2, 2 * BS], BF)  # packed w2 + b2 bias row
    b1_d = din("b1_d", [2, NB, BS])
    fc1w_d = din("fc1w_d", [C, HID], BF)          # pre-scaled by norm2_w
    fc2w_d = din("fc2w_d", [HID, C], BF)
    fc1b_d = din("fc1b_d", [1, HID])              # fc1_b + norm2_b @ fc1_w
    fc2b_d = din("fc2b_d", [1, C], BF)
    n1w_d = din("n1w_d", [1, C], BF)
    n1b_d = din("n1b_d", [1, C], BF)

    out_sh = nc.dram_tensor("out_sh", [RPC, W, C], FP, kind="ExternalOutput").ap()

    s_dram = nc.dram_tensor("s_dram", [RPC, W, C], BF).ap()
    h_dram = nc.dram_tensor("h_dram", [RPC, W, C], BF).ap()
    a2a1_in = nc.dram_tensor("a2a1_in", [NCORES, RPC, KWL, 2, C], BF).ap()
    a2a1_out = nc.dram_tensor("a2a1_out", [NCORES, RPC, KWL, 2, C], BF).ap()
    a2a2_in = nc.dram_tensor("a2a2_in", [NCORES, RPC, KWL, 2, C], BF).ap()
    a2a2_out = nc.dram_tensor("a2a2_out", [NCORES, RPC, KWL, 2, C], BF).ap()

    rg = [list(range(NCORES))]

    with tile.TileContext(nc) as tc:
        with tc.tile_pool(name="cpool", bufs=1) as cp:
            # ---- small constants resident whole kernel ----
            ident = cp.tile([128, 128], BF)
            make_identity(nc, ident[:])
            fw_a = cp.tile([128, 2 * KWP], BF)
            fw_b = cp.tile([52, 2 * KWP], BF)
            nc.sync.dma_start(out=fw_a[:], in_=fw_d[0:128, :])
            nc.sync.dma_start(out=fw_b[:], in_=fw_d[128:180, :])
            fht = cp.tile([H, 2 * H], BF)
            fht2 = cp.tile([H, 2 * H], BF)
            nc.sync.dma_start(out=fht[:], in_=fht_d[:])
            nc.sync.dma_start(out=fht2[:], in_=fht2_d[:])
            cw = cp.tile([2 * KWP, W], BF); nc.sync.dma_start(out=cw[:], in_=cw_d[:])
            blk1 = cp.tile([BS, NB, 3, BS], BF)
            nc.sync.dma_start(out=blk1[:], in_=blk1_d.rearrange("k v i o -> i k v o"))
            w2p = cp.tile([BS + 1, NB, 2, 2 * BS], BF)
            nc.sync.dma_start(out=w2p[:], in_=w2p_d[:])
            b1c = cp.tile([BS, 2, NB], FP)
            nc.sync.dma_start(out=b1c[:], in_=b1_d.rearrange("v k o -> o v k"))
            fc1b = cp.tile([128, 24], FP)
            fc2b_sb = cp.tile([1, C], BF)
            nc.sync.dma_start(out=fc1b[:], in_=fc1b_d.rearrange("x (m p) -> p (x m)", p=128))
            nc.sync.dma_start(out=fc2b_sb[:], in_=fc2b_d[:])
            ones1 = cp.tile([1, 128], BF)
            nc.vector.memset(ones1[:], 1.0)
            # MLP weights, prefetched early so the DMA hides under stage A/middle
            fc1w = cp.tile([128, 6, 24, 128], BF)
            fc2wb = cp.tile([128, 24, C], BF)
            nc.sync.dma_start(out=fc1w[:], in_=fc1w_d.rearrange(
                "(kc p) (m n) -> p kc m n", p=128, n=128))
            nc.sync.dma_start(out=fc2wb[:], in_=fc2w_d.rearrange(
                "(kc p) c -> p kc c", p=128))

            # ================= STAGE A: LN1 + FFT-W per row =================
            with (
                tc.tile_pool(name="sa", bufs=3) as sa,
                tc.tile_pool(name="na", bufs=1) as na,
                tc.tile_pool(name="saps", bufs=2, space="PSUM") as saps,
            ):
                n1w_r = na.tile([128, C], BF)
                n1b_r = na.tile([128, C], BF)
                nc.sync.dma_start(out=n1w_r[:], in_=n1w_d[:].partition_broadcast(128))
                nc.sync.dma_start(out=n1b_r[:], in_=n1b_d[:].partition_broadcast(128))
                for r in range(RPC):
                    h1bf = []
                    for ci, (w0, wn) in enumerate(WCH):
                        xa = sa.tile([128, C], FP, tag=f"xa{ci}", name=f"xa_{r}_{ci}")
                        nc.sync.dma_start(out=xa[:wn], in_=x_sh[r, w0:w0 + wn, :])
                        xbf = sa.tile([128, C], BF, tag=f"xbf{ci}", name=f"xbf_{r}_{ci}")
                        nc.scalar.copy(xbf[:wn], xa[:wn])
                        st = sa.tile([128, 2, 6], FP, tag=f"st{ci}", name=f"st_{r}_{ci}")
                        nc.vector.bn_stats(st[:wn, 0, :], xbf[:wn, 0:384])
                        nc.vector.bn_stats(st[:wn, 1, :], xbf[:wn, 384:768])
                        ag = sa.tile([128, 2], FP, tag=f"ag{ci}", name=f"ag_{r}_{ci}")
                        nc.vector.bn_aggr(ag[:wn], st[:wn])
                        ve = sa.tile([128, 1], FP, tag=f"ve{ci}", name=f"ve_{r}_{ci}")
                        nc.vector.tensor_scalar_add(ve[:wn], ag[:wn, 1:2], LN_EPS)
                        sd = sa.tile([128, 1], FP, tag=f"sd{ci}", name=f"sd_{r}_{ci}")
                        nc.scalar.activation(sd[:wn], ve[:wn], AF.Sqrt)
                        rs = sa.tile([128, 1], FP, tag=f"rs{ci}", name=f"rs_{r}_{ci}")
                        nc.vector.reciprocal(rs[:wn], sd[:wn])
                        t1 = sa.tile([128, C], BF, tag=f"t1{ci}", name=f"t1_{r}_{ci}")
                        nc.vector.scalar_tensor_tensor(
                            t1[:wn], xbf[:wn], ag[:wn, 0:1], n1w_r[:wn],
                            ALU.subtract, ALU.mult)
                        hb = sa.tile([128, C], BF, tag=f"hb{ci}", name=f"hb_{r}_{ci}")
                        nc.vector.scalar_tensor_tensor(
                            hb[:wn], t1[:wn], rs[:wn], n1b_r[:wn],
                            ALU.mult, ALU.add)
                        sv = sa.tile([128, C], BF, tag=f"sv{ci}", name=f"sv_{r}_{ci}")
                        nc.gpsimd.tensor_add(sv[:wn], hb[:wn], xbf[:wn])
                        nc.sync.dma_start(out=s_dram[r, w0:w0 + wn, :], in_=sv[:wn])
                        h1bf.append(hb)
                    for si, (c0, cn) in enumerate(CS):
                        psy = saps.tile([2 * KWP, 384], FP, tag="psy", name=f"psy_{r}_{si}")
                        nc.tensor.matmul(psy[:], fw_a[:], h1bf[0][:, c0:c0 + cn],
                                         start=True, stop=False)
                        nc.tensor.matmul(psy[:], fw_b[:], h1bf[1][0:52, c0:c0 + cn],
                                         start=False, stop=True)
                        yb = sa.tile([2 * KWP, 384], BF, tag="yb", name=f"yb_{r}_{si}")
                        nc.scalar.copy(yb[:], psy[:])
                        for comp in range(2):
                            dst = a2a1_in[:, r, :, comp, c0:c0 + cn]
                            nc.sync.dma_start(out=dst,
                                              in_=yb[comp * KWP:(comp + 1) * KWP, :])

            nc.gpsimd.collective_compute(
                "AllToAll", ALU.bypass, replica_groups=rg,
                ins=[a2a1_in[:]], outs=[a2a1_out[:]])

            # ================= MIDDLE: FFT-H, block MM, iFFT-H =================
            # FFT-H runs data-stationary (lhsT = spectra), so Z lands c-major
            # [c-block, kh] with no transposes; block-MM-2 runs data-stationary
            # (lhsT = o1 chunks), so O2 lands kh-major for the iFFT-H.
            v1 = a2a1_out.rearrange("s r kwl comp c -> (s r) kwl comp c")
            v2in = a2a2_in.rearrange("s r kwl comp c -> (s r) kwl comp c")
            with tc.tile_pool(name="mz", bufs=1) as mz:
                zsb = mz.tile([BS, NB, 2, NTOK], BF)       # [i, k, comp, (b kwl kh)]
                o1sb = mz.tile([BS + 1, NB, 2, NTOK], BF)  # row 96 = ones (bias row)
                nc.vector.memset(o1sb[BS:BS + 1, :, :, :], 1.0)
                with (
                    tc.tile_pool(name="m1", bufs=2) as m1p,
                    tc.tile_pool(name="m1ps", bufs=4, space="PSUM") as m1ps,
                ):
                    for b in range(B):
                        ybt = m1p.tile([H, KWL, 2, C], BF, tag="ybt", name=f"ybt_{b}")
                        nc.sync.dma_start(out=ybt[:], in_=v1[b * H:(b + 1) * H])
                        for kwl in range(KWL):
                            tk0 = (b * KWL + kwl) * H
                            for k in range(NB):
                                psz = m1ps.tile([BS, 2, H], FP, tag="psz",
                                                name=f"psz_{b}_{kwl}_{k}")
                                yr = ybt[:, kwl, 0, k * BS:(k + 1) * BS]
                                yi = ybt[:, kwl, 1, k * BS:(k + 1) * BS]
                                nc.tensor.matmul(psz[:], yr, fht[:], start=True, stop=False)
                                nc.tensor.matmul(psz[:], yi, fht2[:], start=False, stop=True)
                                nc.scalar.copy(zsb[:, k, :, tk0:tk0 + H], psz[:])
                with (
                    tc.tile_pool(name="m2ps", bufs=2, space="PSUM") as m2ps,
                ):
                    for k in range(NB):
                        for t0, tn in BCH:
                            p1r = m2ps.tile([BS, 512], FP, tag="p1r", name=f"p1r_{k}_{t0}")
                            p1i = m2ps.tile([BS, 512], FP, tag="p1i", name=f"p1i_{k}_{t0}")
                            zr = zsb[:, k, 0, t0:t0 + tn]
                            zi = zsb[:, k, 1, t0:t0 + tn]
                            nc.tensor.matmul(p1r[:, :tn], blk1[:, k, 0], zr, start=True, stop=False)
                            nc.tensor.matmul(p1r[:, :tn], blk1[:, k, 2], zi, start=False, stop=True)
                            nc.tensor.matmul(p1i[:, :tn], blk1[:, k, 1], zr, start=True, stop=False)
                            nc.tensor.matmul(p1i[:, :tn], blk1[:, k, 0], zi, start=False, stop=True)
                            nc.scalar.activation(o1sb[0:BS, k, 0, t0:t0 + tn], p1r[:, :tn],
                                                 AF.Relu, bias=b1c[:, 0, k:k + 1])
                            nc.scalar.activation(o1sb[0:BS, k, 1, t0:t0 + tn], p1i[:, :tn],
                                                 AF.Relu, bias=b1c[:, 1, k:k + 1])
                with (
                    tc.tile_pool(name="m3", bufs=2) as m3p,
                    tc.tile_pool(name="m3ps", bufs=2, space="PSUM") as m3ps,
                    tc.tile_pool(name="m3ps2", bufs=2, space="PSUM") as m3ps2,
                ):
                    for b in range(B):
                        for kwl in range(KWL):
                            tk0 = (b * KWL + kwl) * H
                            usb = m3p.tile([H, 2, C], BF, tag="usb",
                                           name=f"usb_{b}_{kwl}")
                            for k in range(NB):
                                pso = m3ps.tile([H, 2, BS], FP, tag="pso",
                                                name=f"pso_{b}_{kwl}_{k}")
                                nc.tensor.matmul(pso[:], o1sb[:, k, 0, tk0:tk0 + H],
                                                 w2p[:, k, 0, :], start=True, stop=False)
                                nc.tensor.matmul(pso[:], o1sb[:, k, 1, tk0:tk0 + H],
                                                 w2p[:, k, 1, :], start=False, stop=True)
                                cl = m3p.tile([H, 2, BS], FP, tag="cl",
                                              name=f"cl_{b}_{kwl}_{k}")
                                nc.vector.tensor_scalar(cl[:], pso[:], -LAM, LAM,
                                                        ALU.max, ALU.min)
                                nc.vector.tensor_sub(usb[:, :, k * BS:(k + 1) * BS],
                                                     pso[:], cl[:])
                            for si, (c0, cn) in enumerate(CS):
                                pur = m3ps2.tile([H, 384], FP, tag="pur",
                                                 name=f"pur_{b}_{kwl}_{si}")
                                pui = m3ps2.tile([H, 384], FP, tag="pui",
                                                 name=f"pui_{b}_{kwl}_{si}")
                                ur_ = usb[:, 0, c0:c0 + cn]
                                ui_ = usb[:, 1, c0:c0 + cn]
                                nc.tensor.matmul(pur[:], fht[:, 0:H], ur_, start=True, stop=False)
                                nc.tensor.matmul(pur[:], fht[:, H:2 * H], ui_, start=False, stop=True)
                                nc.tensor.matmul(pui[:], fht2[:, 0:H], ur_, start=True, stop=False)
                                nc.tensor.matmul(pui[:], fht[:, 0:H], ui_, start=False, stop=True)
                                ub = m3p.tile([H, 2, 384], BF, tag="ub",
                                              name=f"ub_{b}_{kwl}_{si}")
                                nc.scalar.copy(ub[:, 0], pur[:])
                                nc.scalar.copy(ub[:, 1], pui[:])
                                nc.sync.dma_start(
                                    out=v2in[b * H:(b + 1) * H, kwl, 0, c0:c0 + cn],
                                    in_=ub[:, 0])
                                nc.sync.dma_start(
                                    out=v2in[b * H:(b + 1) * H, kwl, 1, c0:c0 + cn],
                                    in_=ub[:, 1])

            nc.gpsimd.collective_compute(
                "AllToAll", ALU.bypass, replica_groups=rg,
                ins=[a2a2_in[:]], outs=[a2a2_out[:]])

            # ================= STAGE B: iFFT-W + LN2 + MLP =================
            with tc.tile_pool(name="sbB", bufs=1) as sbB:
                mlp_in = sbB.tile([128, 6, TOK], BF)
                with (
                    tc.tile_pool(name="b1p", bufs=3) as b1p,
                    tc.tile_pool(name="b1ps", bufs=2, space="PSUM") as b1ps,
                    tc.tile_pool(name="b1pt", bufs=3, space="PSUM") as b1pt,
                ):
                    for r in range(RPC):
                        usb2 = b1p.tile([2 * KWP, C], BF, tag="usb2", name=f"usb2_{r}")
                        for comp in range(2):
                            nc.sync.dma_start(
                                out=usb2[comp * KWP:(comp + 1) * KWP, :],
                                in_=a2a2_out[:, r, :, comp, :])
                        for ci, (w0, wn) in enumerate(WCH):
                            ht = b1p.tile([128, C], BF, tag=f"ht{ci}", name=f"ht_{r}_{ci}")
                            stile = b1p.tile([128, C], BF, tag=f"stl{ci}", name=f"stl_{r}_{ci}")
                            nc.sync.dma_start(out=stile[:wn], in_=s_dram[r, w0:w0 + wn, :])
                            for si, (c0, cn) in enumerate(CS):
                                py = b1ps.tile([128, 384], FP, tag="py",
                                               name=f"py_{r}_{ci}_{si}")
                                nc.tensor.matmul(py[:wn], cw[:, w0:w0 + wn],
                                                 usb2[:, c0:c0 + cn], start=True, stop=True)
                                nc.vector.tensor_add(ht[:wn, c0:c0 + cn], py[:wn],
                                                     stile[:wn, c0:c0 + cn])
                            nc.sync.dma_start(out=h_dram[r, w0:w0 + wn, :], in_=ht[:wn])
                            # LN2 on bf16 ht; norm2 affine is folded into fc1w/fc1b
                            st = b1p.tile([128, 2, 6], FP, tag=f"st{ci}", name=f"bst_{r}_{ci}")
                            nc.vector.bn_stats(st[:wn, 0, :], ht[:wn, 0:384])
                            nc.vector.bn_stats(st[:wn, 1, :], ht[:wn, 384:768])
                            ag = b1p.tile([128, 2], FP, tag=f"ag{ci}", name=f"bag_{r}_{ci}")
                            nc.vector.bn_aggr(ag[:wn], st[:wn])
                            ve = b1p.tile([128, 1], FP, tag=f"ve{ci}", name=f"bve_{r}_{ci}")
                            nc.vector.tensor_scalar_add(ve[:wn], ag[:wn, 1:2], LN_EPS)
                            sd = b1p.tile([128, 1], FP, tag=f"sd{ci}", name=f"bsd_{r}_{ci}")
                            nc.scalar.activation(sd[:wn], ve[:wn], AF.Sqrt)
                            rs = b1p.tile([128, 1], FP, tag=f"rs{ci}", name=f"brs_{r}_{ci}")
                            nc.vector.reciprocal(rs[:wn], sd[:wn])
                            h2 = b1p.tile([128, C], BF, tag=f"h2{ci}", name=f"bh2_{r}_{ci}")
                            nc.vector.tensor_scalar(h2[:wn], ht[:wn], ag[:wn, 0:1], rs[:wn],
                                                    ALU.subtract, ALU.mult)
                            # transpose h2 [wn, 768] -> mlp_in[:, kc, r*W + w0 : +wn]
                            tok0 = r * W + w0
                            for kc in range(6):
                                pt = b1pt.tile([128, 128], BF, tag="pt",
                                               name=f"pt_{r}_{ci}_{kc}")
                                nc.tensor.transpose(pt[:, :wn],
                                                    h2[:wn, kc * 128:(kc + 1) * 128],
                                                    ident[:wn, :wn])
                                nc.scalar.copy(mlp_in[:, kc, tok0:tok0 + wn],
                                               pt[:, :wn])
                # ---- MLP over token chunks; fc2 data-stationary ----
                hv = h_dram.rearrange("r w c -> (r w) c")
                ov = out_sh.rearrange("r w c -> (r w) c")
                with (
                    tc.tile_pool(name="b2p", bufs=3) as b2p,
                    tc.tile_pool(name="gp", bufs=2) as gp,
                    tc.tile_pool(name="b2ps", bufs=3, space="PSUM") as b2ps,
                    tc.tile_pool(name="b2psg", bufs=3, space="PSUM") as b2psg,
                ):
                    for t0 in range(0, TOK, 512):
                        tn = min(512, TOK - t0)
                        gsb = gp.tile([128, 24, 512], BF, tag="gsb", name=f"gsb_{t0}")
                        for m in range(24):
                            pg = b2psg.tile([128, 512], FP, tag="pg", name=f"pg_{t0}_{m}")
                            for kc in range(6):
                                nc.tensor.matmul(pg[:, :tn], fc1w[:, kc, m],
                                                 mlp_in[:, kc, t0:t0 + tn],
                                                 start=(kc == 0), stop=(kc == 5))
                            nc.scalar.activation(gsb[:, m, :tn], pg[:, :tn], AF.Gelu,
                                                 bias=fc1b[:, m:m + 1])
                        for ts0 in range(0, tn, 128):
                            tsn = min(128, tn - ts0)
                            hht = b2p.tile([128, C], BF, tag="hht", name=f"hht_{t0}_{ts0}")
                            nc.sync.dma_start(out=hht[:tsn],
                                              in_=hv[t0 + ts0:t0 + ts0 + tsn, :])
                            outt = b2p.tile([128, C], FP, tag="outt", name=f"outt_{t0}_{ts0}")
                            for si, (c0, cn) in enumerate(CS):
                                po = b2ps.tile([128, 384], FP, tag="po",
                                               name=f"po_{t0}_{ts0}_{si}")
                                for kc in range(24):
                                    nc.tensor.matmul(po[:tsn], gsb[:, kc, ts0:ts0 + tsn],
                                                     fc2wb[:, kc, c0:c0 + cn],
                                                     start=(kc == 0), stop=False)
                                nc.tensor.matmul(po[:tsn], ones1[0:1, 0:tsn],
                                                 fc2b_sb[0:1, c0:c0 + cn],
                                                 start=False, stop=True)
                                nc.vector.tensor_add(outt[:tsn, c0:c0 + cn], po[:tsn],
                                                     hht[:tsn, c0:c0 + cn])
                            nc.sync.dma_start(out=ov[t0 + ts0:t0 + ts0 + tsn, :],
                                              in_=outt[:tsn])
    nc.compile()
    return nc


def _prep_inputs(inputs):
    consts = _dft_consts()
    x = np.asarray(inputs["x"], np.float32)
    xp = np.zeros((RPAD, W, C), np.float32)
    xp[:ROWS] = x.reshape(ROWS, W, C)
    w1 = np.asarray(inputs["w1"], np.float32)
    w2 = np.asarray(inputs["w2"], np.float32)
    b2 = np.asarray(inputs["b2"], np.float32)
    blk1 = np.stack([w1[0], w1[1], -w1[1]], axis=1).astype(bfloat16)  # [8,3,96,96]
    # packed w2 for the data-stationary second block MM:
    # rhs[comp=0] = [w2r | w2i] with bias row [b2r | b2i]; rhs[comp=1] = [-w2i | w2r], 0
    w2p = np.zeros((BS + 1, NB, 2, 2 * BS), np.float32)
    for k in range(NB):
        w2p[0:BS, k, 0, 0:BS] = w2[0][k]
        w2p[0:BS, k, 0, BS:2 * BS] = w2[1][k]
        w2p[BS, k, 0, 0:BS] = b2[0][k]
        w2p[BS, k, 0, BS:2 * BS] = b2[1][k]
        w2p[0:BS, k, 1, 0:BS] = -w2[1][k]
        w2p[0:BS, k, 1, BS:2 * BS] = w2[0][k]
    f32 = lambda k: np.ascontiguousarray(np.asarray(inputs[k], np.float32))
    # fold norm2 affine into fc1
    n2w, n2b = f32("norm2_w"), f32("norm2_b")
    fc1w = f32("fc1_w") * n2w[:, None]
    fc1b = f32("fc1_b") + n2b @ f32("fc1_w")
    common = dict(
        fw_d=consts["fw"], fht_d=consts["fht"], fht2_d=consts["fht2"],
        cw_d=consts["cw"],
        blk1_d=blk1, w2p_d=w2p.astype(bfloat16),
        b1_d=f32("b1"),
        fc1w_d=fc1w.astype(bfloat16), fc2w_d=f32("fc2_w").astype(bfloat16),
        fc1b_d=np.ascontiguousarray(fc1b.reshape(1, HID)),
        fc2b_d=f32("fc2_b").reshape(1, C).astype(bfloat16),
        n1w_d=f32("norm1_w").reshape(1, C).astype(bfloat16),
        n1b_d=f32("norm1_b").reshape(1, C).astype(bfloat16),
    )
    in_maps = []
    for q in range(NCORES):
        m = dict(common)
        m["x_sh"] = np.ascontiguousarray(xp[q * RPC:(q + 1) * RPC])
        in_maps.append(m)
    return in_maps


last_exec_time_ns = None


def kernel(**inputs):
    global last_exec_time_ns
    bass_utils.upload_artifacts = lambda tmpdir: ""  # avoid bucket upload hang under trace
    if "nc" not in _cache:
        _cache["nc"] = _build()
    nc = _cache["nc"]
    in_maps = _prep_inputs(inputs)
    res = bass_utils.run_bass_kernel_spmd(
        nc, in_maps, core_ids=list(range(NCORES)),
        trace=bool(int(__import__("os").environ.get("KERNEL_TRACE", "0"))))
    last_exec_time_ns = res.exec_time_ns
    out = np.concatenate([res.results[q]["out_sh"] for q in range(NCORES)], axis=0)
    return np.ascontiguousarray(out[:ROWS].reshape(B, H, W, C))
